# revision 25
# baseline (speedup 1.0000x reference)
"""Trainium2 Bass kernel for ContrastHead (softnn contrastive KNN loss).

The workload is bound by HBM random-read drain of the SWDGE neighbor gather
(~40ns per 256B descriptor per DMA engine, ~103GB/s/NC effective). Design:

- Table rows pair-pack TWO points per 256B row (features only, f16), so int16
  gather indices need just 2 windows (<=32767 rows each).
- Points with cnt==0 or cnt==K are pruned on the host (loss weight 0): ~15%
  fewer gather descriptors and less DVE work.
- Survivors are sorted globally by window-0 neighbor count and dealt to
  (core, tile) round-robin (tile g -> core g%8), so the per-tile-index
  cross-core slot maxes are tight (~3% padding).
- dist^2 = |f_i|^2 + |f_j|^2 - 2 f_i.f_j: the per-slot norm sum nn is
  host-precomputed, the kernel gathers neighbor features and computes the dot
  against self features pre-scaled by 2 (saves the subtract pass of the
  diff-square formulation).
- Parity (which half of the pair row) is one DVE copy_predicated on i32
  views; then mult/tree-add/reduce -> dot per slot; phase 2 does
  d2 = nn - dot, sqrt, exp with a constant offset (exp(-(d-10)/T): the
  offset cancels in the pos/neg ratio, so no per-point max pass), softmax
  sums per point; one batched ln + cnt-mask at the end.
- Phase 2 is split into blocks so the bulk reduces while later gathers run;
  only a 2-tile remainder lands in the serial tail.
- The idx upload is chunked (scalar-engine HWDGE, parallel to the sync
  uploads) so the first gather starts immediately.
Host sums the 8x(128,2) outputs: loss = -(sum lg)/max(cnt,1).
"""

import os

import numpy as np

import concourse.bacc as bacc
import concourse.bass as bass
import concourse.mybir as mybir
import concourse.tile as tile
from concourse import bass_utils

F16 = mybir.dt.float16
F32 = mybir.dt.float32
I16 = mybir.dt.int16
I32 = mybir.dt.int32

N = 100000
K = 31
C = 64
NPAIR = N // 2              # 50000 pair rows, 256B each
WINR = 32766                # real pair-rows per window
WSTR = WINR + 1             # window stride (incl dummy row 0)
NWIN = 2
NCORES = 8
TEMP = 0.1
EPS = 1e-8
DUMMY = 30.0                # dummy-row feature value -> dist large -> exp 0
NN_PAD = 1.0e6              # nn sentinel for pad slots
DOFF = 10.0                 # exp offset: exp(-(d-DOFF)/T); cancels in the
                            # pos/neg ratio, keeps f32 exponents in range

SINGLE_PACKET = os.environ.get("KSP", "0") == "1"

_CACHE = {}


def _build(k0g, k1g):
    """k0g/k1g: per-tile window-0/1 slot counts (len TPC), cross-core maxes."""
    nc = bacc.Bacc("TRN2", target_bir_lowering=False, debug=False,
                   num_swdge_queues=4)
    TPC = len(k0g)
    kp = [a + b for a, b in zip(k0g, k1g)]
    kmax = max(kp)
    qa_of = [t % 4 for t in range(TPC)]
    qb_of = [(t + 2) % 4 for t in range(TPC)]

    tabT = nc.dram_tensor("tab", (NWIN * WSTR, 128), F16, kind="ExternalInput")
    selfT = nc.dram_tensor("selftab", (128, TPC, C), F16, kind="ExternalInput")
    bounds = [0]
    for a, b in zip(k0g, k1g):
        bounds.append(bounds[-1] + 8 * (a + b))     # idx col offset per tile
    # phase-2 block boundaries: bulk first, 2-tile serial tail
    b0 = max(1, int(TPC * 0.60))
    b1 = max(b0 + 1, TPC - 2)
    # idx upload chunks: first ones small so gathers start early
    tile_ch = [0, 1, 3, 8, 16, 28, 42, 58, TPC]
    tile_ch = sorted(set(min(x, TPC) for x in tile_ch))
    if tile_ch[-1] != TPC:
        tile_ch.append(TPC)
    NCH = len(tile_ch) - 1
    splits = [bounds[b] for b in tile_ch]
    ch_of = []
    for ci in range(NCH):
        ch_of += [ci] * (tile_ch[ci + 1] - tile_ch[ci])
    idxT = nc.dram_tensor("nidx16", (128, bounds[-1]), I16,
                          kind="ExternalInput")
    parT = nc.dram_tensor("par", (128, TPC, kmax), I16, kind="ExternalInput")
    pmT = nc.dram_tensor("pm", (128, TPC, kmax), F32, kind="ExternalInput")
    nnT = nc.dram_tensor("nn", (128, TPC, kmax), F32, kind="ExternalInput")
    outT = nc.dram_tensor("out", (128, 2), F32, kind="ExternalOutput")
    BL = [(0, b0), (b0, b1), (b1, TPC)]

    with tile.TileContext(nc) as tc:
        with (
            tc.tile_pool(name="res", bufs=1) as res,
            tc.tile_pool(name="gpool", bufs=8) as gpool,
            tc.tile_pool(name="p2", bufs=1) as p2,
        ):
            # warmup gathers: one tiny gather per queue (dummy row 0) right
            # at program start, so the SWDGE descriptor-ring + DMA queue
            # initialization overlaps the idx upload instead of delaying the
            # first real gather
            idxz = res.tile([128, 8], I16)
            nc.vector.memset(idxz[:], 0)
            for q in range(4):
                gwu = res.tile([128, 1, 128], F16, tag=f"gwu{q}")
                nc.gpsimd.dma_gather(
                    out_ap=gwu[:, 0:1, :],
                    in_ap=tabT.ap()[0:WSTR, :],
                    idxs_ap=idxz[:, 0:8],
                    num_idxs=128,
                    num_idxs_reg=128,
                    elem_size=128,
                    single_packet=SINGLE_PACKET,
                    queue_num=q,
                )

            idxchunks = []
            for ci in range(NCH):
                a, b = splits[ci], splits[ci + 1]
                ch = res.tile([128, b - a], I16, tag=f"idx{ci}")
                nc.sync.dma_start(out=ch[:], in_=idxT.ap()[:, a:b])
                idxchunks.append((a, ch))
            parsb = res.tile([128, TPC, kmax], I16)
            nc.sync.dma_start(out=parsb[:], in_=parT.ap())
            selfsb = res.tile([128, TPC, C], F16)
            nc.sync.dma_start(out=selfsb[:], in_=selfT.ap())
            pmsb = res.tile([128, TPC, kmax], F32)
            nc.sync.dma_start(out=pmsb[:], in_=pmT.ap())
            nnsb = res.tile([128, TPC, kmax], F32)
            nc.sync.dma_start(out=nnsb[:], in_=nnT.ap())

            dotblocks = []
            for bi, (lo, hi) in enumerate(BL):
                dt_ = res.tile([128, hi - lo, kmax], F32, tag=f"dot_{bi}")
                nc.vector.memset(dt_[:], 0.0)
                dotblocks.append(dt_)
            eps_t = p2.tile([128, 1], F32)
            nc.vector.memset(eps_t[:], EPS)
            chalf = p2.tile([128, 1], F32)
            nc.vector.memset(chalf[:], 0.5)
            ckm = p2.tile([128, 1], F32)
            nc.vector.memset(ckm[:], float(K) - 0.5)
            doff_t = p2.tile([128, 1], F32)
            nc.vector.memset(doff_t[:], DOFF / TEMP)
            outsb = p2.tile([128, 2], F32)
            ratios = p2.tile([128, TPC], F32)
            # cnt mask for all tiles up-front (overlaps the gathers)
            maall = p2.tile([128, TPC], F32)
            cnts = p2.tile([128, TPC], F32)
            nc.vector.reduce_sum(
                out=cnts[:], in_=pmsb[:], axis=mybir.AxisListType.X
            )
            mb2 = p2.tile([128, TPC], F32)
            nc.vector.tensor_tensor(
                out=maall[:], in0=cnts[:],
                in1=chalf[:].broadcast_to([128, TPC]),
                op=mybir.AluOpType.is_gt,
            )
            nc.vector.tensor_tensor(
                out=mb2[:], in0=cnts[:],
                in1=ckm[:].broadcast_to([128, TPC]),
                op=mybir.AluOpType.is_lt,
            )
            nc.vector.tensor_tensor(
                out=maall[:], in0=maall[:], in1=mb2[:],
                op=mybir.AluOpType.mult,
            )

            def gather_range(g, t, idxsb, base, win, sa, sb, qn):
                """Gather slots [sa, sb) of tile t (slot-major idx layout)."""
                nc.gpsimd.dma_gather(
                    out_ap=g[:, sa:sb, :],
                    in_ap=tabT.ap()[win * WSTR : (win + 1) * WSTR, :],
                    idxs_ap=idxsb[:, base + 8 * sa : base + 8 * sb],
                    num_idxs=128 * (sb - sa),
                    num_idxs_reg=128 * (sb - sa),
                    elem_size=128,
                    single_packet=SINGLE_PACKET,
                    queue_num=qn,
                )

            def dot_range(g, t, sa, sb, bi):
                """Parity-select + dot for slots [sa, sb) of tile t."""
                n = sb - sa
                pbc = parsb[:, t, sa:sb].unsqueeze(2).broadcast_to(
                    [128, n, C // 2]
                )
                nc.vector.copy_predicated(
                    out=g[:, sa:sb, 0:C].bitcast(I32),
                    mask=pbc,
                    data=g[:, sa:sb, C : 2 * C].bitcast(I32),
                )
                # dot' = sum(g * 2*self) (selftab pre-scaled by 2 on host)
                fb = selfsb[:, t, :].unsqueeze(1).broadcast_to([128, n, C])
                nc.vector.tensor_tensor(
                    out=g[:, sa:sb, 0:C], in0=g[:, sa:sb, 0:C], in1=fb,
                    op=mybir.AluOpType.mult,
                )
                nc.vector.tensor_add(
                    out=g[:, sa:sb, 0:32], in0=g[:, sa:sb, 0:32],
                    in1=g[:, sa:sb, 32:64],
                )
                nc.vector.tensor_add(
                    out=g[:, sa:sb, 0:16], in0=g[:, sa:sb, 0:16],
                    in1=g[:, sa:sb, 16:32],
                )
                nc.vector.tensor_add(
                    out=g[:, sa:sb, 0:8], in0=g[:, sa:sb, 0:8],
                    in1=g[:, sa:sb, 8:16],
                )
                nc.vector.reduce_sum(
                    out=dotblocks[bi][:, t - BL[bi][0], sa:sb],
                    in_=g[:, sa:sb, 0:8],
                    axis=mybir.AxisListType.X,
                )

            def do_tile(t):
                k0 = k0g[t]
                k1 = k1g[t]
                kpt = k0 + k1
                base = bounds[t]
                ci = ch_of[t]
                choff, idxsb = idxchunks[ci]
                base -= choff
                g = gpool.tile([128, kmax, 128], F16, tag="g")
                bi = next(i for i, (lo, hi) in enumerate(BL) if t < hi)
                if k0 > 0:
                    gather_range(g, t, idxsb, base, 0, 0, k0, qa_of[t])
                if k1 > 0:
                    gather_range(g, t, idxsb, base, 1, k0, kpt, qb_of[t])
                dot_range(g, t, 0, kpt, bi)

            def phase2(dotb, lo, hi, col):
                nt = hi - lo
                pms = pmsb[:, lo:hi, :]
                dist2 = dotb
                # d2 = nn - dot'
                nc.vector.tensor_tensor(
                    out=dist2[:], in0=nnsb[:, lo:hi, :], in1=dotb[:],
                    op=mybir.AluOpType.subtract,
                )
                nc.scalar.sqrt(out=dist2[:], in_=dist2[:])
                # exp(-(d - DOFF)/T): constant offset cancels in pos/neg
                nc.scalar.activation(
                    out=dist2[:], in_=dist2[:],
                    func=mybir.ActivationFunctionType.Exp,
                    scale=-1.0 / TEMP, bias=doff_t[:],
                )
                negs = p2.tile([128, nt], F32, tag=f"negs{col}")
                nc.vector.reduce_sum(
                    out=negs[:], in_=dist2[:], axis=mybir.AxisListType.X
                )
                nc.vector.tensor_tensor(
                    out=dist2[:], in0=dist2[:], in1=pms,
                    op=mybir.AluOpType.mult,
                )
                poss = p2.tile([128, nt], F32, tag=f"poss{col}")
                nc.vector.reduce_sum(
                    out=poss[:], in_=dist2[:], axis=mybir.AxisListType.X
                )
                rn = p2.tile([128, nt], F32, tag=f"rn{col}")
                nc.vector.reciprocal(out=rn[:], in_=negs[:])
                nc.vector.tensor_tensor(
                    out=ratios[:, lo:hi], in0=poss[:], in1=rn[:],
                    op=mybir.AluOpType.mult,
                )

            for bi, (lo, hi) in enumerate(BL):
                for t in range(lo, hi):
                    do_tile(t)
                phase2(dotblocks[bi], lo, hi, 2 * bi)
            # final: lg = ln(ratio + eps) * ma summed, plus the kept count
            lg = p2.tile([128, TPC], F32)
            nc.scalar.activation(
                out=lg[:], in_=ratios[:],
                func=mybir.ActivationFunctionType.Ln, bias=eps_t[:],
            )
            nc.vector.tensor_tensor(
                out=lg[:], in0=lg[:], in1=maall[:], op=mybir.AluOpType.mult
            )
            nc.vector.reduce_sum(
                out=outsb[:, 0:1], in_=lg[:], axis=mybir.AxisListType.X
            )
            nc.vector.reduce_sum(
                out=outsb[:, 1:2], in_=maall[:], axis=mybir.AxisListType.X
            )
            nc.sync.dma_start(out=outT.ap(), in_=outsb[:])

    nc.compile()
    return nc


def _pack_table(features: np.ndarray) -> np.ndarray:
    pairs = features.astype(np.float16).reshape(NPAIR, 2 * C)
    tab = np.zeros((NWIN * WSTR, 2 * C), dtype=np.float16)
    tab[0] = DUMMY
    tab[WSTR] = DUMMY
    tab[1 : 1 + WINR] = pairs[0:WINR]
    tab[WSTR + 1 : WSTR + 1 + (NPAIR - WINR)] = pairs[WINR:NPAIR]
    return tab


def _wrap_idx(vals):
    """vals (128, kcols) slot-major per partition -> SWDGE int16 layout."""
    n = vals.shape[1] * 128
    flat = vals.T.reshape(n)                       # slot i = j*128 + p
    wrapped = flat.reshape(n // 16, 16).T          # (16, n/16)
    return np.tile(wrapped, (8, 1)).astype(np.int16)


def _host_prep(features, labels, neighbor_idx):
    """Prune, globally sort by w0-count, deal tiles to cores round-robin."""
    posmask = labels[:, None] == labels[neighbor_idx]      # (N, K) bool
    cnt = posmask.sum(axis=1)
    kept = np.nonzero((cnt > 0) & (cnt < K))[0]            # (P,)

    nbr = neighbor_idx[kept].astype(np.int64)              # (P, K)
    pos = posmask[kept]
    prow = nbr >> 1
    par = (nbr & 1).astype(np.int16)
    inw1 = prow >= WINR
    c0 = (~inw1).sum(axis=1).astype(np.int64)

    order = np.argsort(c0, kind="stable")
    kept = kept[order]
    prow = prow[order]
    par = par[order]
    pos = pos[order]
    inw1 = inw1[order]
    c0 = c0[order]

    # neighbor-sort each point's slots: w0 first, then w1
    perm = np.argsort(inw1, axis=1, kind="stable")
    prow = np.take_along_axis(prow, perm, axis=1)
    par = np.take_along_axis(par, perm, axis=1)
    pos = np.take_along_axis(pos, perm, axis=1)

    P = len(kept)
    G = (P + 127) // 128
    TPC = (G + NCORES - 1) // NCORES
    PTOT = TPC * NCORES * 128
    padn = PTOT - P
    prow = np.concatenate([prow, np.zeros((padn, K), np.int64)])
    par = np.concatenate([par, np.zeros((padn, K), np.int16)])
    pos = np.concatenate([pos, np.zeros((padn, K), np.bool_)])
    c0 = np.concatenate([c0, np.zeros(padn, np.int64)])
    real = np.concatenate([np.ones(P, np.bool_), np.zeros(padn, np.bool_)])

    normsq = (features.astype(np.float64) ** 2).sum(axis=1).astype(np.float32)
    selfn = np.concatenate(
        [normsq[kept], np.zeros(padn, np.float32)]
    )
    feat_s = np.concatenate(
        [(2.0 * features[kept]).astype(np.float16),
         np.zeros((padn, C), np.float16)]
    )
    # neighbor norm per (padded) point/slot, in sorted-slot order
    jorig = (prow << 1) | par                              # (PTOT, K)
    nbrn = normsq[np.clip(jorig, 0, N - 1)]                # (PTOT, K) f32

    return {
        "prow": prow, "par": par, "pos": pos.astype(np.float32),
        "c0": c0, "real": real, "selfn": selfn, "nbrn": nbrn,
        "feat": feat_s, "TPC": TPC,
    }


def _core_point_rows(prep, core):
    """Global row indices (into the padded sorted arrays) for this core's
    points, laid out (TPC, 128): tile g -> core g%8, per-core index g//8."""
    TPC = prep["TPC"]
    g = core + NCORES * np.arange(TPC)                     # global tile ids
    base = g[:, None] * 128 + np.arange(128)[None, :]      # (TPC, 128)
    return base


def _tile_sizes(prep):
    """Cross-core per-tile-index k0/k1 maxes."""
    TPC = prep["TPC"]
    c0 = prep["c0"]
    real = prep["real"]
    c0r = np.where(real, c0, 0)
    c1r = np.where(real, K - c0, 0)
    k0g = np.zeros(TPC, np.int64)
    k1g = np.zeros(TPC, np.int64)
    for core in range(NCORES):
        rows = _core_point_rows(prep, core)                # (TPC, 128)
        k0g = np.maximum(k0g, c0r[rows].max(axis=1))
        k1g = np.maximum(k1g, c1r[rows].max(axis=1))
    return [int(v) for v in k0g], [int(v) for v in k1g]


def _core_inputs(table, prep, core, k0g, k1g):
    TPC = prep["TPC"]
    kp = [a + b for a, b in zip(k0g, k1g)]
    kmax = max(kp)
    rows = _core_point_rows(prep, core)                    # (TPC, 128)
    idx_cols = np.zeros((128, 8 * sum(kp)), dtype=np.int16)
    par_t = np.zeros((128, TPC, kmax), dtype=np.int16)
    pm_t = np.zeros((128, TPC, kmax), dtype=np.float32)
    nn_t = np.full((128, TPC, kmax), NN_PAD, dtype=np.float32)

    prow = prep["prow"]
    par = prep["par"]
    pos = prep["pos"]
    c0 = prep["c0"]
    real = prep["real"]
    selfn = prep["selfn"]
    nbrn = prep["nbrn"]

    base = 0
    for t in range(TPC):
        r = rows[t]                                        # (128,) row ids
        rl = real[r]
        c0p = np.where(rl, c0[r], 0)                       # (128,)
        c1p = np.where(rl, K - c0[r], 0)
        k0 = k0g[t]
        k1 = k1g[t]
        kpt = k0 + k1

        prow_p = prow[r]                                   # (128, K)
        par_p = par[r]
        pos_p = pos[r]
        nbrn_p = nbrn[r]
        nnself = selfn[r]

        jj = np.arange(k0)
        m0 = jj[None, :] < c0p[:, None]                    # (128, k0)
        idx0 = np.where(m0, prow_p[:, :k0] + 1, 0).astype(np.int16)
        par_t[:, t, :k0] = np.where(m0, par_p[:, :k0], 0)
        pm_t[:, t, :k0] = np.where(m0, pos_p[:, :k0], 0)
        nn_t[:, t, :k0] = np.where(
            m0, nnself[:, None] + nbrn_p[:, :k0], NN_PAD
        )

        sidx = c0p[:, None] + np.arange(k1)[None, :]       # (128, k1)
        valid = np.arange(k1)[None, :] < c1p[:, None]
        sc = np.clip(sidx, 0, K - 1)
        g1 = np.take_along_axis(prow_p, sc, axis=1)
        idx1 = np.where(valid, g1 - WINR + 1, 0).astype(np.int16)
        par_t[:, t, k0:kpt] = np.where(
            valid, np.take_along_axis(par_p, sc, axis=1), 0
        )
        pm_t[:, t, k0:kpt] = np.where(
            valid, np.take_along_axis(pos_p, sc, axis=1), 0
        )
        nn_t[:, t, k0:kpt] = np.where(
            valid,
            nnself[:, None] + np.take_along_axis(nbrn_p, sc, axis=1),
            NN_PAD,
        )

        # pad points (no real slots) would give negs=0 -> 1/0 -> NaN; give
        # them a benign nn so exp is a normal small float (pm=0 -> ratio=0)
        nn_t[~rl, t, :] = 144.0

        idx_cols[:, base : base + 8 * k0] = _wrap_idx(idx0)
        idx_cols[:, base + 8 * k0 : base + 8 * kpt] = _wrap_idx(idx1)
        base += 8 * kpt

    feat = prep["feat"]
    rows_all = rows.reshape(-1)
    selftab = np.ascontiguousarray(
        feat[rows_all].reshape(TPC, 128, C).transpose(1, 0, 2)
    )
    return {
        "tab": table,
        "selftab": selftab,
        "nidx16": idx_cols,
        "par": par_t,
        "pm": pm_t,
        "nn": nn_t,
    }


def run(features, labels, neighbor_idx, trace=False):
    features = np.asarray(features)
    labels = np.asarray(labels)
    neighbor_idx = np.asarray(neighbor_idx)

    table = _pack_table(features)
    prep = _host_prep(features, labels, neighbor_idx)
    k0g, k1g = _tile_sizes(prep)

    key = (tuple(k0g), tuple(k1g), SINGLE_PACKET)
    if _CACHE.get("key") != key:
        _CACHE["nc"] = _build(k0g, k1g)
        _CACHE["key"] = key
    nc = _CACHE["nc"]

    in_maps = [
        _core_inputs(table, prep, c, k0g, k1g) for c in range(NCORES)
    ]
    res = bass_utils.run_bass_kernel_spmd(
        nc, in_maps, core_ids=list(range(NCORES)), trace=trace
    )
    s = 0.0
    ccnt = 0.0
    for o in res.results:
        out = o["out"].astype(np.float64)
        s += float(out[:, 0].sum())
        ccnt += float(out[:, 1].sum())
    loss = np.float32(-s / max(ccnt, 1.0))
    return loss, res


def kernel(features, labels, neighbor_idx):
    loss, _ = run(features, labels, neighbor_idx, trace=False)
    return loss


# revision 26
# speedup vs baseline: 1.0194x; 1.0194x over previous
"""Trainium2 Bass kernel for ContrastHead (softnn contrastive KNN loss).

The workload is bound by HBM random-read drain of the SWDGE neighbor gather
(~40ns per 256B descriptor per DMA engine, ~103GB/s/NC effective). Design:

- Table rows pair-pack TWO points per 256B row (features only, f16), so int16
  gather indices need just 2 windows (<=32767 rows each).
- Points with cnt==0 or cnt==K are pruned on the host (loss weight 0): ~15%
  fewer gather descriptors and less DVE work.
- Survivors are sorted globally by window-0 neighbor count and dealt to
  (core, tile) round-robin (tile g -> core g%8), so the per-tile-index
  cross-core slot maxes are tight (~3% padding).
- dist^2 = |f_i|^2 + |f_j|^2 - 2 f_i.f_j: the per-slot norm sum nn is
  host-precomputed, the kernel gathers neighbor features and computes the dot
  against self features pre-scaled by 2 (saves the subtract pass of the
  diff-square formulation).
- Parity (which half of the pair row) is one DVE copy_predicated on i32
  views; then mult/tree-add/reduce -> dot per slot; phase 2 does
  d2 = nn - dot, sqrt, exp with a constant offset (exp(-(d-10)/T): the
  offset cancels in the pos/neg ratio, so no per-point max pass), softmax
  sums per point; one batched ln + cnt-mask at the end.
- Phase 2 is split into blocks so the bulk reduces while later gathers run;
  only a 2-tile remainder lands in the serial tail.
- The idx upload is chunked (scalar-engine HWDGE, parallel to the sync
  uploads) so the first gather starts immediately.
Host sums the 8x(128,2) outputs: loss = -(sum lg)/max(cnt,1).
"""

import os

import numpy as np

import concourse.bacc as bacc
import concourse.bass as bass
import concourse.mybir as mybir
import concourse.tile as tile
from concourse import bass_utils

F16 = mybir.dt.float16
F32 = mybir.dt.float32
I16 = mybir.dt.int16
I32 = mybir.dt.int32

N = 100000
K = 31
C = 64
NPAIR = N // 2              # 50000 pair rows, 256B each
WINR = 32766                # real pair-rows per window
WSTR = WINR + 1             # window stride (incl dummy row 0)
NWIN = 2
NCORES = 8
TEMP = 0.1
EPS = 1e-8
DUMMY = 30.0                # dummy-row feature value -> dist large -> exp 0
NN_PAD = 1.0e6              # nn sentinel for pad slots
DOFF = 10.0                 # exp offset: exp(-(d-DOFF)/T); cancels in the
                            # pos/neg ratio, keeps f32 exponents in range

SINGLE_PACKET = os.environ.get("KSP", "0") == "1"

_CACHE = {}


def _build(k0g, k1g):
    """k0g/k1g: per-tile window-0/1 slot counts (len TPC), cross-core maxes."""
    nc = bacc.Bacc("TRN2", target_bir_lowering=False, debug=False,
                   num_swdge_queues=4)
    TPC = len(k0g)
    kp = [a + b for a, b in zip(k0g, k1g)]
    kmax = max(kp)
    qa_of = [t % 4 for t in range(TPC)]
    qb_of = [(t + 2) % 4 for t in range(TPC)]

    tabT = nc.dram_tensor("tab", (NWIN * WSTR, 128), F16, kind="ExternalInput")
    selfT = nc.dram_tensor("selftab", (128, TPC, C), F16, kind="ExternalInput")
    bounds = [0]
    for a, b in zip(k0g, k1g):
        bounds.append(bounds[-1] + 8 * (a + b))     # idx col offset per tile
    # phase-2 block boundaries: bulk first, 2-tile serial tail
    b0 = max(1, int(TPC * 0.60))
    b1 = max(b0 + 1, TPC - 2)
    # idx upload chunks: first ones small so gathers start early
    tile_ch = [0, 1, 3, 8, 16, 28, 42, 58, TPC]
    tile_ch = sorted(set(min(x, TPC) for x in tile_ch))
    if tile_ch[-1] != TPC:
        tile_ch.append(TPC)
    NCH = len(tile_ch) - 1
    splits = [bounds[b] for b in tile_ch]
    ch_of = []
    for ci in range(NCH):
        ch_of += [ci] * (tile_ch[ci + 1] - tile_ch[ci])
    idxT = nc.dram_tensor("nidx16", (128, bounds[-1]), I16,
                          kind="ExternalInput")
    parT = nc.dram_tensor("par", (128, TPC, kmax), I16, kind="ExternalInput")
    pmT = nc.dram_tensor("pm", (128, TPC, kmax), F32, kind="ExternalInput")
    nnT = nc.dram_tensor("nn", (128, TPC, kmax), F32, kind="ExternalInput")
    outT = nc.dram_tensor("out", (128, 2), F32, kind="ExternalOutput")
    BL = [(0, b0), (b0, b1), (b1, TPC)]

    with tile.TileContext(nc) as tc:
        with (
            tc.tile_pool(name="res", bufs=1) as res,
            tc.tile_pool(name="gpool", bufs=8) as gpool,
            tc.tile_pool(name="p2", bufs=1) as p2,
        ):
            idxchunks = []
            for ci in range(NCH):
                a, b = splits[ci], splits[ci + 1]
                ch = res.tile([128, b - a], I16, tag=f"idx{ci}")
                nc.sync.dma_start(out=ch[:], in_=idxT.ap()[:, a:b])
                idxchunks.append((a, ch))
            parsb = res.tile([128, TPC, kmax], I16)
            nc.sync.dma_start(out=parsb[:], in_=parT.ap())
            selfsb = res.tile([128, TPC, C], F16)
            nc.sync.dma_start(out=selfsb[:], in_=selfT.ap())
            pmsb = res.tile([128, TPC, kmax], F32)
            nc.sync.dma_start(out=pmsb[:], in_=pmT.ap())
            nnsb = res.tile([128, TPC, kmax], F32)
            nc.sync.dma_start(out=nnsb[:], in_=nnT.ap())

            dotblocks = []
            for bi, (lo, hi) in enumerate(BL):
                dt_ = res.tile([128, hi - lo, kmax], F32, tag=f"dot_{bi}")
                nc.vector.memset(dt_[:], 0.0)
                dotblocks.append(dt_)
            eps_t = p2.tile([128, 1], F32)
            nc.vector.memset(eps_t[:], EPS)
            chalf = p2.tile([128, 1], F32)
            nc.vector.memset(chalf[:], 0.5)
            ckm = p2.tile([128, 1], F32)
            nc.vector.memset(ckm[:], float(K) - 0.5)
            doff_t = p2.tile([128, 1], F32)
            nc.vector.memset(doff_t[:], DOFF / TEMP)
            outsb = p2.tile([128, 2], F32)
            ratios = p2.tile([128, TPC], F32)
            # cnt mask for all tiles up-front (overlaps the gathers)
            maall = p2.tile([128, TPC], F32)
            cnts = p2.tile([128, TPC], F32)
            nc.vector.reduce_sum(
                out=cnts[:], in_=pmsb[:], axis=mybir.AxisListType.X
            )
            mb2 = p2.tile([128, TPC], F32)
            nc.vector.tensor_tensor(
                out=maall[:], in0=cnts[:],
                in1=chalf[:].broadcast_to([128, TPC]),
                op=mybir.AluOpType.is_gt,
            )
            nc.vector.tensor_tensor(
                out=mb2[:], in0=cnts[:],
                in1=ckm[:].broadcast_to([128, TPC]),
                op=mybir.AluOpType.is_lt,
            )
            nc.vector.tensor_tensor(
                out=maall[:], in0=maall[:], in1=mb2[:],
                op=mybir.AluOpType.mult,
            )

            def gather_range(g, t, idxsb, base, win, sa, sb, qn):
                """Gather slots [sa, sb) of tile t (slot-major idx layout)."""
                nc.gpsimd.dma_gather(
                    out_ap=g[:, sa:sb, :],
                    in_ap=tabT.ap()[win * WSTR : (win + 1) * WSTR, :],
                    idxs_ap=idxsb[:, base + 8 * sa : base + 8 * sb],
                    num_idxs=128 * (sb - sa),
                    num_idxs_reg=128 * (sb - sa),
                    elem_size=128,
                    single_packet=SINGLE_PACKET,
                    queue_num=qn,
                )

            def dot_range(g, t, sa, sb, bi):
                """Parity-select + dot for slots [sa, sb) of tile t."""
                n = sb - sa
                pbc = parsb[:, t, sa:sb].unsqueeze(2).broadcast_to(
                    [128, n, C // 2]
                )
                nc.vector.copy_predicated(
                    out=g[:, sa:sb, 0:C].bitcast(I32),
                    mask=pbc,
                    data=g[:, sa:sb, C : 2 * C].bitcast(I32),
                )
                # dot' = sum(g * 2*self) (selftab pre-scaled by 2 on host)
                fb = selfsb[:, t, :].unsqueeze(1).broadcast_to([128, n, C])
                nc.vector.tensor_tensor(
                    out=g[:, sa:sb, 0:C], in0=g[:, sa:sb, 0:C], in1=fb,
                    op=mybir.AluOpType.mult,
                )
                nc.vector.tensor_add(
                    out=g[:, sa:sb, 0:32], in0=g[:, sa:sb, 0:32],
                    in1=g[:, sa:sb, 32:64],
                )
                nc.vector.tensor_add(
                    out=g[:, sa:sb, 0:16], in0=g[:, sa:sb, 0:16],
                    in1=g[:, sa:sb, 16:32],
                )
                nc.vector.tensor_add(
                    out=g[:, sa:sb, 0:8], in0=g[:, sa:sb, 0:8],
                    in1=g[:, sa:sb, 8:16],
                )
                nc.vector.reduce_sum(
                    out=dotblocks[bi][:, t - BL[bi][0], sa:sb],
                    in_=g[:, sa:sb, 0:8],
                    axis=mybir.AxisListType.X,
                )

            def do_tile(t):
                k0 = k0g[t]
                k1 = k1g[t]
                kpt = k0 + k1
                base = bounds[t]
                ci = ch_of[t]
                choff, idxsb = idxchunks[ci]
                base -= choff
                g = gpool.tile([128, kmax, 128], F16, tag="g")
                bi = next(i for i, (lo, hi) in enumerate(BL) if t < hi)
                if k0 > 0:
                    gather_range(g, t, idxsb, base, 0, 0, k0, qa_of[t])
                if k1 > 0:
                    gather_range(g, t, idxsb, base, 1, k0, kpt, qb_of[t])
                dot_range(g, t, 0, kpt, bi)

            def phase2(dotb, lo, hi, col):
                nt = hi - lo
                pms = pmsb[:, lo:hi, :]
                dist2 = dotb
                # d2 = nn - dot'
                nc.vector.tensor_tensor(
                    out=dist2[:], in0=nnsb[:, lo:hi, :], in1=dotb[:],
                    op=mybir.AluOpType.subtract,
                )
                nc.scalar.sqrt(out=dist2[:], in_=dist2[:])
                # exp(-(d - DOFF)/T): constant offset cancels in pos/neg
                nc.scalar.activation(
                    out=dist2[:], in_=dist2[:],
                    func=mybir.ActivationFunctionType.Exp,
                    scale=-1.0 / TEMP, bias=doff_t[:],
                )
                negs = p2.tile([128, nt], F32, tag=f"negs{col}")
                nc.vector.reduce_sum(
                    out=negs[:], in_=dist2[:], axis=mybir.AxisListType.X
                )
                nc.vector.tensor_tensor(
                    out=dist2[:], in0=dist2[:], in1=pms,
                    op=mybir.AluOpType.mult,
                )
                poss = p2.tile([128, nt], F32, tag=f"poss{col}")
                nc.vector.reduce_sum(
                    out=poss[:], in_=dist2[:], axis=mybir.AxisListType.X
                )
                rn = p2.tile([128, nt], F32, tag=f"rn{col}")
                nc.vector.reciprocal(out=rn[:], in_=negs[:])
                nc.vector.tensor_tensor(
                    out=ratios[:, lo:hi], in0=poss[:], in1=rn[:],
                    op=mybir.AluOpType.mult,
                )

            for bi, (lo, hi) in enumerate(BL):
                for t in range(lo, hi):
                    do_tile(t)
                phase2(dotblocks[bi], lo, hi, 2 * bi)
            # final: lg = ln(ratio + eps) * ma summed, plus the kept count
            lg = p2.tile([128, TPC], F32)
            nc.scalar.activation(
                out=lg[:], in_=ratios[:],
                func=mybir.ActivationFunctionType.Ln, bias=eps_t[:],
            )
            nc.vector.tensor_tensor(
                out=lg[:], in0=lg[:], in1=maall[:], op=mybir.AluOpType.mult
            )
            nc.vector.reduce_sum(
                out=outsb[:, 0:1], in_=lg[:], axis=mybir.AxisListType.X
            )
            nc.vector.reduce_sum(
                out=outsb[:, 1:2], in_=maall[:], axis=mybir.AxisListType.X
            )
            nc.sync.dma_start(out=outT.ap(), in_=outsb[:])

    nc.compile()
    return nc


def _pack_table(features: np.ndarray) -> np.ndarray:
    pairs = features.astype(np.float16).reshape(NPAIR, 2 * C)
    tab = np.zeros((NWIN * WSTR, 2 * C), dtype=np.float16)
    tab[0] = DUMMY
    tab[WSTR] = DUMMY
    tab[1 : 1 + WINR] = pairs[0:WINR]
    tab[WSTR + 1 : WSTR + 1 + (NPAIR - WINR)] = pairs[WINR:NPAIR]
    return tab


def _wrap_idx(vals):
    """vals (128, kcols) slot-major per partition -> SWDGE int16 layout."""
    n = vals.shape[1] * 128
    flat = vals.T.reshape(n)                       # slot i = j*128 + p
    wrapped = flat.reshape(n // 16, 16).T          # (16, n/16)
    return np.tile(wrapped, (8, 1)).astype(np.int16)


def _host_prep(features, labels, neighbor_idx):
    """Prune, globally sort by w0-count, deal tiles to cores round-robin."""
    posmask = labels[:, None] == labels[neighbor_idx]      # (N, K) bool
    cnt = posmask.sum(axis=1)
    kept = np.nonzero((cnt > 0) & (cnt < K))[0]            # (P,)

    nbr = neighbor_idx[kept].astype(np.int64)              # (P, K)
    pos = posmask[kept]
    prow = nbr >> 1
    par = (nbr & 1).astype(np.int16)
    inw1 = prow >= WINR
    c0 = (~inw1).sum(axis=1).astype(np.int64)

    order = np.argsort(c0, kind="stable")
    kept = kept[order]
    prow = prow[order]
    par = par[order]
    pos = pos[order]
    inw1 = inw1[order]
    c0 = c0[order]

    # neighbor-sort each point's slots: w0 first, then w1
    perm = np.argsort(inw1, axis=1, kind="stable")
    prow = np.take_along_axis(prow, perm, axis=1)
    par = np.take_along_axis(par, perm, axis=1)
    pos = np.take_along_axis(pos, perm, axis=1)

    P = len(kept)
    G = (P + 127) // 128
    TPC = (G + NCORES - 1) // NCORES
    PTOT = TPC * NCORES * 128
    padn = PTOT - P
    prow = np.concatenate([prow, np.zeros((padn, K), np.int64)])
    par = np.concatenate([par, np.zeros((padn, K), np.int16)])
    pos = np.concatenate([pos, np.zeros((padn, K), np.bool_)])
    c0 = np.concatenate([c0, np.zeros(padn, np.int64)])
    real = np.concatenate([np.ones(P, np.bool_), np.zeros(padn, np.bool_)])

    normsq = (features.astype(np.float64) ** 2).sum(axis=1).astype(np.float32)
    selfn = np.concatenate(
        [normsq[kept], np.zeros(padn, np.float32)]
    )
    feat_s = np.concatenate(
        [(2.0 * features[kept]).astype(np.float16),
         np.zeros((padn, C), np.float16)]
    )
    # neighbor norm per (padded) point/slot, in sorted-slot order
    jorig = (prow << 1) | par                              # (PTOT, K)
    nbrn = normsq[np.clip(jorig, 0, N - 1)]                # (PTOT, K) f32

    return {
        "prow": prow, "par": par, "pos": pos.astype(np.float32),
        "c0": c0, "real": real, "selfn": selfn, "nbrn": nbrn,
        "feat": feat_s, "TPC": TPC,
    }


def _core_point_rows(prep, core):
    """Global row indices (into the padded sorted arrays) for this core's
    points, laid out (TPC, 128): tile g -> core g%8, per-core index g//8."""
    TPC = prep["TPC"]
    g = core + NCORES * np.arange(TPC)                     # global tile ids
    base = g[:, None] * 128 + np.arange(128)[None, :]      # (TPC, 128)
    return base


def _tile_sizes(prep):
    """Cross-core per-tile-index k0/k1 maxes."""
    TPC = prep["TPC"]
    c0 = prep["c0"]
    real = prep["real"]
    c0r = np.where(real, c0, 0)
    c1r = np.where(real, K - c0, 0)
    k0g = np.zeros(TPC, np.int64)
    k1g = np.zeros(TPC, np.int64)
    for core in range(NCORES):
        rows = _core_point_rows(prep, core)                # (TPC, 128)
        k0g = np.maximum(k0g, c0r[rows].max(axis=1))
        k1g = np.maximum(k1g, c1r[rows].max(axis=1))
    return [int(v) for v in k0g], [int(v) for v in k1g]


def _core_inputs(table, prep, core, k0g, k1g):
    TPC = prep["TPC"]
    kp = [a + b for a, b in zip(k0g, k1g)]
    kmax = max(kp)
    rows = _core_point_rows(prep, core)                    # (TPC, 128)
    idx_cols = np.zeros((128, 8 * sum(kp)), dtype=np.int16)
    par_t = np.zeros((128, TPC, kmax), dtype=np.int16)
    pm_t = np.zeros((128, TPC, kmax), dtype=np.float32)
    nn_t = np.full((128, TPC, kmax), NN_PAD, dtype=np.float32)

    prow = prep["prow"]
    par = prep["par"]
    pos = prep["pos"]
    c0 = prep["c0"]
    real = prep["real"]
    selfn = prep["selfn"]
    nbrn = prep["nbrn"]

    base = 0
    for t in range(TPC):
        r = rows[t]                                        # (128,) row ids
        rl = real[r]
        c0p = np.where(rl, c0[r], 0)                       # (128,)
        c1p = np.where(rl, K - c0[r], 0)
        k0 = k0g[t]
        k1 = k1g[t]
        kpt = k0 + k1

        prow_p = prow[r]                                   # (128, K)
        par_p = par[r]
        pos_p = pos[r]
        nbrn_p = nbrn[r]
        nnself = selfn[r]

        jj = np.arange(k0)
        m0 = jj[None, :] < c0p[:, None]                    # (128, k0)
        idx0 = np.where(m0, prow_p[:, :k0] + 1, 0).astype(np.int16)
        par_t[:, t, :k0] = np.where(m0, par_p[:, :k0], 0)
        pm_t[:, t, :k0] = np.where(m0, pos_p[:, :k0], 0)
        nn_t[:, t, :k0] = np.where(
            m0, nnself[:, None] + nbrn_p[:, :k0], NN_PAD
        )

        sidx = c0p[:, None] + np.arange(k1)[None, :]       # (128, k1)
        valid = np.arange(k1)[None, :] < c1p[:, None]
        sc = np.clip(sidx, 0, K - 1)
        g1 = np.take_along_axis(prow_p, sc, axis=1)
        idx1 = np.where(valid, g1 - WINR + 1, 0).astype(np.int16)
        par_t[:, t, k0:kpt] = np.where(
            valid, np.take_along_axis(par_p, sc, axis=1), 0
        )
        pm_t[:, t, k0:kpt] = np.where(
            valid, np.take_along_axis(pos_p, sc, axis=1), 0
        )
        nn_t[:, t, k0:kpt] = np.where(
            valid,
            nnself[:, None] + np.take_along_axis(nbrn_p, sc, axis=1),
            NN_PAD,
        )

        # pad points (no real slots) would give negs=0 -> 1/0 -> NaN; give
        # them a benign nn so exp is a normal small float (pm=0 -> ratio=0)
        nn_t[~rl, t, :] = 144.0

        idx_cols[:, base : base + 8 * k0] = _wrap_idx(idx0)
        idx_cols[:, base + 8 * k0 : base + 8 * kpt] = _wrap_idx(idx1)
        base += 8 * kpt

    feat = prep["feat"]
    rows_all = rows.reshape(-1)
    selftab = np.ascontiguousarray(
        feat[rows_all].reshape(TPC, 128, C).transpose(1, 0, 2)
    )
    return {
        "tab": table,
        "selftab": selftab,
        "nidx16": idx_cols,
        "par": par_t,
        "pm": pm_t,
        "nn": nn_t,
    }


def run(features, labels, neighbor_idx, trace=False):
    features = np.asarray(features)
    labels = np.asarray(labels)
    neighbor_idx = np.asarray(neighbor_idx)

    table = _pack_table(features)
    prep = _host_prep(features, labels, neighbor_idx)
    k0g, k1g = _tile_sizes(prep)

    key = (tuple(k0g), tuple(k1g), SINGLE_PACKET)
    if _CACHE.get("key") != key:
        _CACHE["nc"] = _build(k0g, k1g)
        _CACHE["key"] = key
    nc = _CACHE["nc"]

    in_maps = [
        _core_inputs(table, prep, c, k0g, k1g) for c in range(NCORES)
    ]
    res = bass_utils.run_bass_kernel_spmd(
        nc, in_maps, core_ids=list(range(NCORES)), trace=trace
    )
    s = 0.0
    ccnt = 0.0
    for o in res.results:
        out = o["out"].astype(np.float64)
        s += float(out[:, 0].sum())
        ccnt += float(out[:, 1].sum())
    loss = np.float32(-s / max(ccnt, 1.0))
    return loss, res


def kernel(features, labels, neighbor_idx):
    loss, _ = run(features, labels, neighbor_idx, trace=False)
    return loss
